# revision 1
# baseline (speedup 1.0000x reference)
"""HSTU block-sparse attention (cmp + slc branches) on 8 Trainium2 cores.

Sharding: the 32 (batch, head) pairs are split 4-per-core (core c gets
b = c // 2, heads 4*(c % 2) .. 4*(c % 2)+3). Each core runs the full
per-(b,h) pipeline: block-mean k/v compression, gate matmul + sigmoid,
compressed-branch SiLU attention, causal top-16 block selection (max8 +
match_replace), and the selected-branch SiLU attention, all fused in one
Bass/Tile module. Host side only scatters jagged->dense (gather_idx),
packs per-core operand layouts, and gathers the jagged output back.
"""

import sys

sys.path.insert(0, "/opt/trn_rl_repo")

import numpy as np
import ml_dtypes

B, N, H, D = 4, 1024, 8, 64
BLOCK_SIZE = 32
NB = N // BLOCK_SIZE          # 32 blocks
NQT = N // 128                # 8 query tiles of 128
PAIRS = 4                     # (b,h) pairs per core
NCORES = 8
SCALE = D ** -0.5
MINVAL = -1.0e30
BIGRAW = 1.0e6                # additive mask bias (pre-scale); silu saturates to 0

_CACHE = {}


def _build_statics():
    if "statics" in _CACHE:
        return _CACHE["statics"]
    bf = ml_dtypes.bfloat16
    ident = np.eye(128, dtype=np.float32)
    i32b = np.eye(32, dtype=bf)
    i128b = np.eye(128, dtype=bf)
    # e32[j, key] = 1 if key // 32 == j (block expansion over the full key axis)
    key = np.arange(N)
    e32 = (key[None, :] // BLOCK_SIZE == np.arange(NB)[:, None]).astype(bf)
    # dbias[key j, q i] = 0 if i >= j else -BIGRAW (intra-tile token causal)
    i_q = np.arange(128)
    dbias = np.where(i_q[None, :] >= i_q[:, None], 0.0, -BIGRAW).astype(bf)
    # cmpcaus[blk, t, i] = 0 if blk <= qblk(128 t + i) else -BIGRAW
    qblk = (128 * np.arange(NQT)[:, None] + i_q[None, :]) // BLOCK_SIZE  # [t, i]
    blk = np.arange(NB)
    cmpcaus = np.where(blk[:, None, None] <= qblk[None, :, :], 0.0, -BIGRAW).astype(bf)
    # selcaus[i, j, blk] = +1e30 if blk <= qblk(128 (4+j) + i) else MINVAL
    selcaus = np.where(blk[None, None, :] <= qblk[4:].T[:, :, None],
                       1.0e30, MINVAL).astype(np.float32)
    # mred[q, i, blk] = 1/32 if 4 i + q // 32 == blk else 0
    gblk = (np.arange(NQT)[None, :] * 4 + (i_q // BLOCK_SIZE)[:, None])  # [q, i]
    mred = (gblk[:, :, None] == blk[None, None, :]).astype(np.float32) / BLOCK_SIZE
    mredb = mred.astype(bf)
    statics = {
        "ident": ident, "i32b": i32b, "i128b": i128b, "e32": e32,
        "dbias": dbias, "cmpcaus": cmpcaus, "selcaus": selcaus,
        "mred": mred, "mredb": mredb,
    }
    _CACHE["statics"] = statics
    return statics


def _build_nc():
    if "nc" in _CACHE:
        return _CACHE["nc"]
    import concourse.bacc as bacc
    import concourse.mybir as mybir
    from concourse.tile import TileContext

    F32 = mybir.dt.float32
    BF16 = mybir.dt.bfloat16
    AF = mybir.ActivationFunctionType
    OP = mybir.AluOpType

    nc = bacc.Bacc("TRN2", target_bir_lowering=False, debug=False,
                   num_devices=NCORES)

    d_qT = nc.dram_tensor("qT", [PAIRS, 64, N], BF16, kind="ExternalInput")
    d_kT = nc.dram_tensor("kT", [PAIRS, 64, N], BF16, kind="ExternalInput")
    d_pqT = nc.dram_tensor("pqT", [PAIRS, 64, N], F32, kind="ExternalInput")
    d_vn = nc.dram_tensor("vn", [PAIRS, N, 64], BF16, kind="ExternalInput")
    d_pkn = nc.dram_tensor("pkn", [PAIRS, N, 64], F32, kind="ExternalInput")
    d_pvn = nc.dram_tensor("pvn", [PAIRS, N, 64], BF16, kind="ExternalInput")
    d_gw = nc.dram_tensor("gw", [PAIRS, 64, 2], F32, kind="ExternalInput")
    d_cm = nc.dram_tensor("cmpmask", [64, NB], F32, kind="ExternalInput")
    d_id = nc.dram_tensor("ident", [128, 128], F32, kind="ExternalInput")
    d_i32 = nc.dram_tensor("i32b", [32, 32], BF16, kind="ExternalInput")
    d_i128 = nc.dram_tensor("i128b", [128, 128], BF16, kind="ExternalInput")
    d_e32 = nc.dram_tensor("e32", [NB, N], BF16, kind="ExternalInput")
    d_db = nc.dram_tensor("dbias", [128, 128], BF16, kind="ExternalInput")
    d_cc = nc.dram_tensor("cmpcaus", [NB, NQT, 128], BF16, kind="ExternalInput")
    d_sc = nc.dram_tensor("selcaus", [128, 4, NB], F32, kind="ExternalInput")
    d_mr = nc.dram_tensor("mred", [128, NQT, NB], F32, kind="ExternalInput")
    d_mrb = nc.dram_tensor("mredb", [128, NQT, NB], BF16, kind="ExternalInput")
    d_out = nc.dram_tensor("out", [PAIRS, N, 64], F32, kind="ExternalOutput")

    with TileContext(nc) as tc:
        with tc.tile_pool(name="sb_c", bufs=1) as sb_c, \
             tc.tile_pool(name="sb_io", bufs=2) as sb_io, \
             tc.tile_pool(name="sb_w", bufs=3) as sb_w, \
             tc.tile_pool(name="ps_st", bufs=2, space="PSUM") as ps_st, \
             tc.tile_pool(name="ps_os", bufs=2, space="PSUM") as ps_os, \
             tc.tile_pool(name="ps_misc", bufs=2, space="PSUM") as ps_misc, \
             tc.tile_pool(name="ps_pre", bufs=2, space="PSUM") as ps_pre:

            t_id = sb_c.tile([128, 128], F32, tag="t_id")
            nc.sync.dma_start(t_id[:], d_id[:])
            t_i32 = sb_c.tile([32, 32], BF16, tag="t_i32")
            nc.sync.dma_start(t_i32[:], d_i32[:])
            t_i128 = sb_c.tile([128, 128], BF16, tag="t_i128")
            nc.sync.dma_start(t_i128[:], d_i128[:])
            t_e32 = sb_c.tile([NB, N], BF16, tag="t_e32")
            nc.sync.dma_start(t_e32[:], d_e32[:])
            t_db = sb_c.tile([128, 128], BF16, tag="t_db")
            nc.sync.dma_start(t_db[:], d_db[:])
            t_cc = sb_c.tile([NB, NQT, 128], BF16, tag="t_cc")
            nc.sync.dma_start(t_cc[:], d_cc[:])
            t_sc = sb_c.tile([128, 4, NB], F32, tag="t_sc")
            nc.sync.dma_start(t_sc[:], d_sc[:])
            t_mr = sb_c.tile([128, NQT, NB], F32, tag="t_mr")
            nc.sync.dma_start(t_mr[:], d_mr[:])
            t_mrb = sb_c.tile([128, NQT, NB], BF16, tag="t_mrb")
            nc.sync.dma_start(t_mrb[:], d_mrb[:])
            t_cm = sb_c.tile([64, NB], F32, tag="t_cm")
            nc.sync.dma_start(t_cm[:], d_cm[:])

            for p in range(PAIRS):
                t_q = sb_io.tile([64, N], BF16, tag="t_q")
                nc.sync.dma_start(t_q[:], d_qT[p])
                t_k = sb_io.tile([64, N], BF16, tag="t_k")
                nc.sync.dma_start(t_k[:], d_kT[p])
                t_pq = sb_io.tile([64, N], F32, tag="t_pq")
                nc.sync.dma_start(t_pq[:], d_pqT[p])
                t_v = sb_io.tile([128, NQT, 64], BF16, tag="t_v")
                nc.sync.dma_start(t_v[:], d_vn[p].rearrange("(i q) d -> q i d", q=128))
                t_pk = sb_io.tile([128, NQT, 64], F32, tag="t_pk")
                nc.sync.dma_start(t_pk[:], d_pkn[p].rearrange("(i q) d -> q i d", q=128))
                t_pv = sb_io.tile([128, NQT, 64], BF16, tag="t_pv")
                nc.sync.dma_start(t_pv[:], d_pvn[p].rearrange("(i q) d -> q i d", q=128))
                t_gw = sb_io.tile([64, 2], F32, tag="t_gw")
                nc.sync.dma_start(t_gw[:], d_gw[p])

                # ---- k_cmp = block mean of padded_k: [64 d, 32 blk] ----
                p_kc = ps_pre.tile([64, NB], F32, tag="pre")
                for i in range(NQT):
                    nc.tensor.matmul(p_kc[:], lhsT=t_pk[:, i, :], rhs=t_mr[:, i, :],
                                     start=(i == 0), stop=(i == NQT - 1))
                kcf = sb_w.tile([64, NB], F32, tag="kcf")
                nc.scalar.copy(kcf[:], p_kc[:])
                kcb = sb_w.tile([64, NB], BF16, tag="kcb")
                nc.vector.tensor_mul(kcb[:], kcf[:], t_cm[:])
                # ---- v_cmp = block mean of padded_v: [32 blk, 64 d] ----
                p_vc = ps_pre.tile([32, 64], F32, tag="pre")
                for i in range(NQT):
                    nc.tensor.matmul(p_vc[:], lhsT=t_mrb[:, i, :], rhs=t_pv[:, i, :],
                                     start=(i == 0), stop=(i == NQT - 1))
                vcb = sb_w.tile([32, 64], BF16, tag="vcb")
                nc.scalar.copy(vcb[:], p_vc[:])

                # ---- prepass: gates + top-16 block selection bias ----
                g_all = sb_w.tile([128, NQT, 2], F32, tag="g_all")
                selbT = sb_w.tile([NB, NQT, 128], BF16, tag="selbT")
                for t in range(NQT):
                    qs = t_pq[:, 128 * t:128 * (t + 1)]
                    p_g = ps_pre.tile([128, 2], F32, tag="pre")
                    nc.tensor.matmul(p_g[:], lhsT=qs, rhs=t_gw[:], start=True, stop=True)
                    nc.scalar.activation(g_all[:, t, :], p_g[:], AF.Sigmoid)
                    if t >= 4:
                        p_sel = ps_pre.tile([128, NB], F32, tag="pre")
                        nc.tensor.matmul(p_sel[:], lhsT=qs, rhs=kcf[:],
                                         start=True, stop=True)
                        sm = sb_w.tile([128, NB], F32, tag="sm")
                        nc.vector.tensor_tensor(sm[:], p_sel[:], t_sc[:, t - 4, :],
                                                OP.min)
                        mx = sb_w.tile([128, 8], F32, tag="mx")
                        nc.vector.max(mx[:], sm[:])
                        rep = sb_w.tile([128, NB], F32, tag="rep")
                        nc.vector.match_replace(rep[:], mx[:], sm[:], MINVAL)
                        mx2 = sb_w.tile([128, 8], F32, tag="mx2")
                        nc.vector.max(mx2[:], rep[:])
                        rep2 = sb_w.tile([128, NB], F32, tag="rep2")
                        nc.vector.match_replace(rep2[:], mx2[:], rep[:], MINVAL)
                        dif = sb_w.tile([128, NB], F32, tag="dif")
                        nc.vector.tensor_sub(dif[:], sm[:], rep2[:])
                        nc.vector.tensor_scalar_min(dif[:], dif[:], 1.0)
                        bq = sb_w.tile([128, NB], F32, tag="bq")
                        nc.vector.tensor_scalar(bq[:], dif[:], 1.0, BIGRAW,
                                                OP.subtract, OP.mult)
                        p_bt = ps_pre.tile([NB, 128], F32, tag="pre")
                        nc.tensor.transpose(p_bt[:], bq[:], t_id[:])
                        nc.scalar.copy(selbT[:, t, :], p_bt[:])

                # ---- main pass ----
                for t in range(NQT):
                    qsb = t_q[:, 128 * t:128 * (t + 1)]
                    selb = t_cc[:, t, :] if t < 4 else selbT[:, t, :]
                    # compressed branch
                    p_ct = ps_misc.tile([NB, 128], F32, tag="misc")
                    nc.tensor.matmul(p_ct[:], lhsT=kcb[:], rhs=qsb,
                                     start=True, stop=False)
                    nc.tensor.matmul(p_ct[:], lhsT=t_i32[:], rhs=t_cc[:, t, :],
                                     start=False, stop=True)
                    pc = sb_w.tile([NB, 128], BF16, tag="pc")
                    nc.scalar.activation(pc[:], p_ct[:], AF.Silu, scale=SCALE)
                    p_oc = ps_misc.tile([128, 64], F32, tag="misc")
                    nc.tensor.matmul(p_oc[:], lhsT=pc[:], rhs=vcb[:],
                                     start=True, stop=True)
                    # selected branch
                    p_os = ps_os.tile([128, 64], F32, tag="os")
                    for kt in range(t + 1):
                        p_st = ps_st.tile([128, 128], F32, tag="st")
                        nc.tensor.matmul(p_st[:], lhsT=t_k[:, 128 * kt:128 * (kt + 1)],
                                         rhs=qsb, start=True, stop=False)
                        nc.tensor.matmul(p_st[:], lhsT=t_e32[:, 128 * kt:128 * (kt + 1)],
                                         rhs=selb, start=False, stop=(kt != t))
                        if kt == t:
                            nc.tensor.matmul(p_st[:], lhsT=t_i128[:], rhs=t_db[:],
                                             start=False, stop=True)
                        pT = sb_w.tile([128, 128], BF16, tag="pT")
                        nc.scalar.activation(pT[:], p_st[:], AF.Silu, scale=SCALE)
                        nc.tensor.matmul(p_os[:], lhsT=pT[:], rhs=t_v[:, kt, :],
                                         start=(kt == 0), stop=(kt == t))
                    # combine: out = g_cmp * o_cmp + g_slc * o_slc
                    o1 = sb_w.tile([128, 64], F32, tag="o1")
                    nc.scalar.activation(o1[:], p_oc[:], AF.Copy,
                                         scale=g_all[:, t, 0:1])
                    o2 = sb_w.tile([128, 64], F32, tag="o2")
                    nc.vector.tensor_tensor(o2[:], p_os[:],
                                            g_all[:, t, 1:2].to_broadcast([128, 64]),
                                            OP.mult)
                    nc.vector.tensor_add(o2[:], o2[:], o1[:])
                    nc.sync.dma_start(d_out[p, 128 * t:128 * (t + 1), :], o2[:])

    nc.compile()
    _CACHE["nc"] = nc
    return nc


def _get_runner():
    """Persistent jitted 8-core runner (mirrors run_bass_via_pjrt's
    multi-core branch but caches the jit so repeat calls skip recompiles)."""
    if "runner" in _CACHE:
        return _CACHE["runner"]
    import jax
    import numpy as _np
    from jax.experimental.shard_map import shard_map
    from jax.sharding import Mesh, PartitionSpec
    import concourse.mybir as mybir
    from concourse.bass2jax import (_bass_exec_p, install_neuronx_cc_hook,
                                    partition_id_tensor)

    nc = _build_nc()
    install_neuronx_cc_hook()

    partition_name = (nc.partition_id_tensor.name
                      if nc.partition_id_tensor else None)
    in_names, out_names, out_avals, zero_shapes = [], [], [], []
    for alloc in nc.m.functions[0].allocations:
        if not isinstance(alloc, mybir.MemoryLocationSet):
            continue
        name = alloc.memorylocations[0].name
        if alloc.kind == "ExternalInput":
            if name != partition_name:
                in_names.append(name)
        elif alloc.kind == "ExternalOutput":
            shape = tuple(alloc.tensor_shape)
            dtype = mybir.dt.np(alloc.dtype)
            out_names.append(name)
            out_avals.append(jax.core.ShapedArray(shape, dtype))
            zero_shapes.append((shape, dtype))
    n_params = len(in_names)
    all_names = in_names + out_names
    if partition_name is not None:
        all_names = all_names + [partition_name]

    def _body(*args):
        operands = list(args)
        if partition_name is not None:
            operands.append(partition_id_tensor())
        outs = _bass_exec_p.bind(
            *operands,
            out_avals=tuple(out_avals),
            in_names=tuple(all_names),
            out_names=tuple(out_names),
            lowering_input_output_aliases=(),
            sim_require_finite=True,
            sim_require_nnan=True,
            nc=nc,
        )
        return tuple(outs)

    devices = jax.devices()[:NCORES]
    mesh = Mesh(_np.asarray(devices), ("core",))
    n_outs = len(out_names)
    sharded = jax.jit(
        shard_map(_body, mesh=mesh,
                  in_specs=(PartitionSpec("core"),) * (n_params + n_outs),
                  out_specs=(PartitionSpec("core"),) * n_outs,
                  check_rep=False),
        donate_argnums=tuple(range(n_params, n_params + n_outs)),
        keep_unused=True,
    )

    def run(in_maps):
        concat_in = [
            np.concatenate([in_maps[c][name] for c in range(NCORES)], axis=0)
            for name in in_names
        ]
        concat_zeros = [np.zeros((NCORES * s[0], *s[1:]), dt)
                        for s, dt in zero_shapes]
        out_arrs = sharded(*concat_in, *concat_zeros)
        return [
            {name: np.asarray(out_arrs[i]).reshape(NCORES, *out_avals[i].shape)[c]
             for i, name in enumerate(out_names)}
            for c in range(NCORES)
        ]

    _CACHE["runner"] = run
    return run


def _prepare_in_maps(jagged_q, jagged_k, jagged_v, padded_q, padded_k,
                     padded_v, x_offsets, gate_w, gather_idx):
    bf = ml_dtypes.bfloat16
    st = _build_statics()
    gidx = np.asarray(gather_idx).astype(np.int64)

    def to_dense(j):
        d = np.zeros((B * N, H, D), np.float32)
        d[gidx] = np.asarray(j, np.float32)
        return d.reshape(B, N, H, D)

    qd = to_dense(jagged_q)
    kd = to_dense(jagged_k)
    vd = to_dense(jagged_v)
    pq = np.asarray(padded_q, np.float32)
    pk = np.asarray(padded_k, np.float32)
    pv = np.asarray(padded_v, np.float32)
    gw = np.asarray(gate_w, np.float32)
    offs = np.asarray(x_offsets).astype(np.int64)
    lengths = offs[1:] - offs[:-1]
    cmp_len = np.clip((lengths + BLOCK_SIZE - 1) // BLOCK_SIZE, 0, NB)

    in_maps = []
    for c in range(NCORES):
        b = c // 2
        hs = [4 * (c % 2) + j for j in range(PAIRS)]
        qT = np.stack([qd[b, :, h, :].T for h in hs]).astype(bf)
        kT = np.stack([kd[b, :, h, :].T for h in hs]).astype(bf)
        pqT = np.stack([pq[b, :, h, :].T for h in hs]).astype(np.float32)
        vn = np.stack([vd[b, :, h, :] for h in hs]).astype(bf)
        pkn = np.stack([pk[b, :, h, :] for h in hs]).astype(np.float32)
        pvn = np.stack([pv[b, :, h, :] for h in hs]).astype(bf)
        gwc = np.stack([gw[h, :, 0:2] for h in hs]).astype(np.float32)
        cmpmask = np.broadcast_to(
            (np.arange(NB) < cmp_len[b]).astype(np.float32), (64, NB)).copy()
        in_maps.append({
            "qT": np.ascontiguousarray(qT), "kT": np.ascontiguousarray(kT),
            "pqT": np.ascontiguousarray(pqT), "vn": np.ascontiguousarray(vn),
            "pkn": np.ascontiguousarray(pkn), "pvn": np.ascontiguousarray(pvn),
            "gw": np.ascontiguousarray(gwc), "cmpmask": cmpmask,
            "ident": st["ident"], "i32b": st["i32b"], "i128b": st["i128b"],
            "e32": st["e32"], "dbias": st["dbias"], "cmpcaus": st["cmpcaus"],
            "selcaus": st["selcaus"], "mred": st["mred"], "mredb": st["mredb"],
        })
    return in_maps, gidx


def kernel(jagged_q, jagged_k, jagged_v, jagged_u, padded_q, padded_k,
           padded_v, x_offsets, gate_w, padding_mask, gather_idx):
    in_maps, gidx = _prepare_in_maps(jagged_q, jagged_k, jagged_v, padded_q,
                                     padded_k, padded_v, x_offsets, gate_w,
                                     gather_idx)
    run = _get_runner()
    results = run(in_maps)
    o_dense = np.zeros((B, N, H, D), np.float32)
    for c in range(NCORES):
        b = c // 2
        hs = [4 * (c % 2) + j for j in range(PAIRS)]
        out = results[c]["out"]
        for p, h in enumerate(hs):
            o_dense[b, :, h, :] = out[p]
    return o_dense.reshape(B * N, H, D)[gidx]

